# revision 33
# baseline (speedup 1.0000x reference)
"""Trainium2 Bass kernel for the CAM-drop attention module.

Reference computation (per sample n):
    cams  = relu(W @ x[n])            # W: [C=64, Cin=1024], x[n]: [Cin, H*W]
    thr_k = gama * max_hw(cams[k])    # per-channel spatial max
    drop  = where(cams > thr, 0, cams)
    out[n] = x[n] * mean_k(drop)      # broadcast over Cin

Data-parallel over the batch: 32 samples sharded 4-per-core across 8
NeuronCores; fc_weights / gama replicated. No cross-core communication.

The problem is HBM-bound, so x is pre-cast to bf16 on the host and loaded
as bf16, and the output is stored as bf16 and widened to f32 on the host
(halves both HBM streams; rel err stays ~7e-3, well under the 2e-2 gate).
Matmuls accumulate bf16 into f32 PSUM; the channel mean is bf16.

Per-core pipeline (samples unrolled):
  - x[n] streamed as 8 bf16 tiles [128, 3136] into a 30-slot rotating SBUF
    pool (3.75 samples of load prefetch); loads on the sync HWDGE ring,
    consts (w prelaid [128, 512] on host, gama) on the scalar ring so x
    bytes flow from ~8us (framework sem setup is the rest of the ramp)
  - cams accumulated in f32 PSUM with ONE TILE PER BANK (7 tags): PSUM
    dependency tracking at bank granularity, so relu_s chases the final
    matmul pass and sample n+1's matmuls chase the per-bank mean copies
  - per-bank relu (ACT) -> two partial spatial maxes + final (DVE),
    threshold, in-place drop-mask (DVE scalar_tensor_tensor)
  - channel mean via a bf16 [64->128] ones/64 matmul into the same per-bank
    PSUM slots, copied per-bank PSUM->SBUF on ACT
  - products IN PLACE: xb tile *= mean (DVE 2x tensor_tensor), stores read
    the xb tile; tile 0 chunked per bank to chase the copies with its store
    on the scalar HWDGE ring; tiles 1-7 stores on the gpsimd SWDGE ring,
    except the last sample's on the scalar ring (shorter completion drain)
  - host widens the bf16 output back to f32

Measured pitfalls baked into the structure: GpSimd tensor ops running
concurrently with DVE 2x-mode ops contend for SBUF ports and slow both
~4x; dense 4x-mode tensor_scalar activity trips HAM power throttling
(50%-duty windows); ScalarE ACTIVATE has no 16-bit accel (2.9us per
[64, 3136] op) so sign/compare paths stay off ACT.

Steady state is HBM-bus-bound at ~420 GB/s observed per core (51.4 MB
-> ~122us) with the DVE serial stream (~21.9us/sample) setting the
pipeline period and the tail.
"""

import numpy as np

# Problem shape (hardcoded per harness contract).
N, CIN, H, W = 32, 1024, 56, 56
C = 64
HW = H * W          # 3136
NCORES = 8
NS = N // NCORES    # 4 samples per core
P = 128             # SBUF partitions
NT = CIN // P       # 8 Cin tiles
NCH = 7             # spatial chunks per sample
CH = HW // NCH      # 448 (fits one PSUM bank)
BANK = 512          # PSUM bank stride in f32 elements
NBBUF = 30          # rotating bf16 x-tile slots (0.784 MB each)

_CACHE = {}


def _build_nc():
    from concourse import bacc, bass, tile
    from concourse import mybir

    f32 = mybir.dt.float32
    bf16 = mybir.dt.bfloat16
    alu = mybir.AluOpType

    nc = bacc.Bacc("TRN2", target_bir_lowering=False, debug=False)
    x_ext = nc.declare_dram_parameter("x", [NS, CIN, HW], bf16, isOutput=False)
    # fc_weights prelaid on host as [p, t*C+c] = w[c, t*128+p]: contiguous
    # 1KB partition lines -> one efficient DMA (the [CIN, C] layout's 128B
    # lines ran at ~24 GB/s and stalled the load ring for ~10us at startup).
    w_ext = nc.declare_dram_parameter("fc_weights", [P, NT * C], bf16, isOutput=False)
    g_ext = nc.declare_dram_parameter("gama", [C, 2], f32, isOutput=False)
    out_ext = nc.declare_dram_parameter("out", [NS, CIN, HW], bf16, isOutput=True)

    with tile.TileContext(nc) as tc:
        with (
            tc.tile_pool(name="consts", bufs=1) as constp,
            tc.tile_pool(name="xbp", bufs=NBBUF) as xbp,
            tc.tile_pool(name="stats", bufs=2) as statp,
            tc.tile_pool(name="camsb", bufs=1) as camp,
            tc.tile_pool(name="meanp", bufs=1) as meanp,
            tc.tile_pool(name="psum", bufs=1, space=bass.MemorySpace.PSUM) as psump,
        ):
            # Consts go on the scalar HWDGE ring so the sync ring starts
            # streaming x immediately (loads and consts in parallel).
            w_sb = constp.tile([P, NT, C], bf16)
            nc.scalar.dma_start(
                out=w_sb[:].rearrange("p a b -> p (a b)"), in_=w_ext[:, :]
            )
            # Columns: (gama, -gama).
            g_sb = constp.tile([C, 2], f32)
            nc.scalar.dma_start(out=g_sb[:], in_=g_ext[:])
            ones_sb = constp.tile([C, P], bf16)
            nc.vector.memset(ones_sb[:], 1.0 / C)

            # PE clock warm-up: the HAM gate holds the PE at half clock until
            # ~4us of sustained matmul activity. Garbage matmuls into a spare
            # PSUM bank (never read; DCE keeps unread matmuls) warm it up
            # during the initial load-only DMA phase.
            warm_ps = psump.tile([C, BANK], f32, name="warm_ps", tag="warm")
            w_flat = w_sb[:].rearrange("p a b -> p (a b)")
            for _ in range(15):
                nc.tensor.matmul(
                    warm_ps[:, :], w_sb[:, 0, :], w_flat[:, 0:BANK],
                    start=True, stop=True,
                )

            for n in range(NS):
                xbs = []
                for t in range(NT):
                    xb = xbp.tile([P, HW], bf16, name=f"xb_{n}_{t}", tag="xb")
                    nc.sync.dma_start(out=xb[:], in_=x_ext[n, t * P:(t + 1) * P, :])
                    xbs.append(xb)

                # One PSUM tile per bank (tag per bank, bufs=1): dependency
                # tracking at bank granularity. Each bank's slot alternates
                # cams(n) -> mean(n) -> cams(n+1): relu_s fires right after
                # the last matmul touching bank s, and sample n+1's t=0
                # matmul pass chases the per-bank mean copies instead of
                # waiting for all 7 (tile-granular PSUM WAR cost ~10us/sample
                # in the v2 trace).
                cams = [
                    psump.tile([P, BANK], f32, name=f"cams_{n}_{s}", tag=f"bank{s}")
                    for s in range(NCH)
                ]
                crelu = camp.tile([C, NCH, CH], bf16, name=f"crelu_{n}", tag="crelu")
                for t in range(NT):
                    for s in range(NCH):
                        nc.tensor.matmul(
                            cams[s][0:C, 0:CH],
                            w_sb[:, t, :],
                            xbs[t][:, s * CH:(s + 1) * CH],
                            start=(t == 0),
                            stop=(t == NT - 1),
                        )
                # Per-bank relu chases the final (t=NT-1) matmul pass.
                for s in range(NCH):
                    nc.scalar.activation(
                        crelu[:, s, :], cams[s][0:C, 0:CH],
                        mybir.ActivationFunctionType.Relu,
                    )
                # Spatial max in two partials chasing the relus; final max
                # combines. max(crelu) == relu(max(cams)), so thr =
                # max(crelu) * gama directly (and -thr via the -gama col).
                cmax2 = statp.tile([C, 2], f32, name=f"cmax2_{n}", tag="cmax2")
                nc.vector.tensor_reduce(
                    cmax2[:, 0:1], crelu[:, 0:4, :], axis=mybir.AxisListType.XY,
                    op=alu.max,
                )
                nc.vector.tensor_reduce(
                    cmax2[:, 1:2], crelu[:, 4:NCH, :], axis=mybir.AxisListType.XY,
                    op=alu.max,
                )
                cmax = statp.tile([C, 1], f32, name=f"cmax_{n}", tag="cmax")
                nc.vector.tensor_reduce(
                    cmax[:], cmax2[:], axis=mybir.AxisListType.X, op=alu.max
                )
                thr = statp.tile([C, 1], f32, name=f"thr_{n}", tag="thr")
                nc.vector.tensor_scalar(
                    thr[:], cmax[:], g_sb[:, 0:1], None, op0=alu.mult
                )

                # drop = crelu * (crelu <= thr), in place (comparing post-relu
                # values against thr >= 0 matches the reference's pre-relu
                # compare). Then the channel mean, broadcast to all 128
                # partitions via a ones/64 matmul into the per-bank slots.
                mean_ps = [
                    psump.tile([P, BANK], f32, name=f"meanps_{n}_{s}", tag=f"bank{s}")
                    for s in range(NCH)
                ]
                mean_sb = meanp.tile([P, HW], bf16, name=f"mean_{n}", tag="mean")
                mean_sb3 = mean_sb[:].rearrange("p (a b) -> p a b", a=NCH)
                # Mask as the fused scalar_tensor_tensor (1x mode but a
                # single pass): splitting it into a 4x is_le + 2x multiply
                # measured WORSE end-to-end -- the denser 4x op activity
                # trips HAM power throttling (50% duty windows).
                for s0, s1 in ((0, 4), (4, NCH)):
                    nc.vector.scalar_tensor_tensor(
                        crelu[:, s0:s1, :], crelu[:, s0:s1, :], thr[:],
                        crelu[:, s0:s1, :], op0=alu.is_le, op1=alu.mult,
                    )
                for s in range(NCH):
                    nc.tensor.matmul(
                        mean_ps[s][:, 0:CH], ones_sb[:], crelu[:, s, :],
                        start=True, stop=True,
                    )
                for s in range(NCH):
                    nc.scalar.copy(mean_sb3[:, s, :], mean_ps[s][:, 0:CH])

                # Products overwrite the xb tiles in place (no separate out
                # pool -> 6 more xb slots of load prefetch). Tile 0 is
                # chunked per bank so it chases the ACT copies, and its
                # store goes out on the (idle) scalar HWDGE ring. All
                # products stay on DVE: a GpSimd tensor op running
                # concurrently with DVE 2x-mode ops contends for SBUF ports
                # and slows BOTH ~4x (measured 1.78us -> 7.7us).
                xb0 = xbs[0][:].rearrange("p (a b) -> p a b", a=NCH)
                for s in range(NCH):
                    nc.vector.tensor_mul(
                        xb0[:, s, :], xb0[:, s, :], mean_sb3[:, s, :]
                    )
                nc.scalar.dma_start(out=out_ext[n, 0:P, :], in_=xbs[0][:])
                # The last sample's stores alternate between the two HWDGE
                # rings (scalar/sync) -- both engines are idle by then, the
                # rings drain concurrently, and HWDGE completion is ~0.6us
                # vs ~2us SWDGE -- shortening the final drain.
                for t in range(1, NT):
                    nc.vector.tensor_mul(xbs[t][:], xbs[t][:], mean_sb[:])
                    if n == NS - 1:
                        store_eng = nc.scalar if t % 2 else nc.sync
                    else:
                        store_eng = nc.gpsimd
                    store_eng.dma_start(
                        out=out_ext[n, t * P:(t + 1) * P, :], in_=xbs[t][:]
                    )
    nc.compile()
    return nc


def _get_nc():
    if "nc" not in _CACHE:
        _CACHE["nc"] = _build_nc()
    return _CACHE["nc"]


def _make_in_maps(x, fc_weights, gama):
    from concourse import mybir

    bf16_np = mybir.dt.np(mybir.dt.bfloat16)
    x = np.asarray(x, dtype=np.float32)
    # [p, t*C+c] = w[c, t*128+p]: one contiguous [128, 512] block.
    w2 = np.ascontiguousarray(
        np.asarray(fc_weights, dtype=np.float32)
        .reshape(C, NT, P)
        .transpose(2, 1, 0)
        .reshape(P, NT * C)
    ).astype(bf16_np)
    g = np.asarray(gama, dtype=np.float32).reshape(1, 1)
    g64 = np.ascontiguousarray(
        np.broadcast_to(np.concatenate([g, -g], axis=1), (C, 2))
    )
    return [
        {
            "x": np.ascontiguousarray(
                x[i * NS:(i + 1) * NS].reshape(NS, CIN, HW)
            ).astype(bf16_np),
            "fc_weights": w2,
            "gama": g64,
        }
        for i in range(NCORES)
    ]


def kernel(x: np.ndarray, fc_weights: np.ndarray, gama: np.ndarray) -> np.ndarray:
    from concourse.bass_utils import run_bass_kernel_spmd

    nc = _get_nc()
    in_maps = _make_in_maps(x, fc_weights, gama)
    res = run_bass_kernel_spmd(nc, in_maps, core_ids=list(range(NCORES)))
    out = np.concatenate(
        [
            res.results[i]["out"].astype(np.float32).reshape(NS, CIN, H, W)
            for i in range(NCORES)
        ],
        axis=0,
    )
    return out

